# revision 4
# baseline (speedup 1.0000x reference)
"""DeepHisCoM Trainium2 kernel (nn_DeepHisCoM_7017976562218).

Math (reference):
    xr = x.reshape(B, P, V)
    z1 = einsum('bpv,pwv->bpw', xr, W1);  h = leaky(z1)          # per-pathway Linear V->W
    z2 = einsum('bpw,pw->bp', h, W2);     pval = leaky(z2)       # per-pathway Linear W->1
    BN(batch stats) -> global L2 normalize -> sigmoid(pn @ Wd + bd)

Device strategy (8 NeuronCores, batch-sharded 2048 rows/core):
    - For each [128 batch x 128 V] block: TensorE transpose (fp32) so V lands on
      partitions, ScalarE copies PSUM->SBUF casting to bf16.
    - One 66-column matmul per pathway: rhs = [W1p^T | +u | -u] (bf16) where
      u = 0.2 * W1p^T @ W2p.  leaky(z1) = 0.2*z1 + 0.8*relu(z1), so
      z2 = sum_w relu(z1)*0.8*W2 + (relu(q) - relu(-q)) with q = 0.2*sum_w z1*W2
      carried exactly by the +/-u columns through the uniform relu.
    - VectorE: fused (max(h,0) * W2ext) in one scalar_tensor_tensor, then a
      strided reduce -> z2 columns; final leaky via max(0.2*z, z).
    - BN stats + L2 norm + final linear + sigmoid on host (8 MiB, trivial).

bf16 is safe here: the global L2 norm makes the logits tiny, final rel err ~5e-7
(measured against the fp32 reference pipeline).
"""

import os
import sys

import numpy as np

for _p in ("/opt/trn_rl_repo",):
    if _p not in sys.path and os.path.isdir(_p):
        sys.path.insert(0, _p)

import ml_dtypes

import concourse.bacc as bacc
import concourse.bass as bass
import concourse.mybir as mybir
from concourse.bass_utils import run_bass_kernel_spmd
from concourse.tile import TileContext

P, V, W = 128, 128, 64
B = 16384
N_CORES = 8
BSH = B // N_CORES          # 2048 batch rows per core
NBT = BSH // 128            # 16 batch tiles per core
BN_EPS = 1e-5
NCOL = W + 2                # 66: W1^T columns + (+u, -u)
F32 = mybir.dt.float32
BF16 = mybir.dt.bfloat16

# pathway groups per 64-pathway half: (start, size); size split across 2 PSUM banks
GROUPS = [(0, 14), (14, 14), (28, 14), (42, 14), (56, 8)]

_CACHE = {}
LAST_RESULTS = None


def _build_program():
    nc = bacc.Bacc()
    x_in = nc.declare_dram_parameter("xs", [BSH, P * V], F32, isOutput=False)
    wext_in = nc.declare_dram_parameter("wext", [V, P * NCOL], BF16, isOutput=False)
    w2e_in = nc.declare_dram_parameter("w2ext", [128, P * NCOL], F32, isOutput=False)
    id_in = nc.declare_dram_parameter("ident", [128, 128], F32, isOutput=False)
    p_out = nc.declare_dram_parameter("ps", [BSH, P], F32, isOutput=True)

    with TileContext(nc) as tc:
        with (
            tc.tile_pool(name="singles", bufs=1) as singles,
            tc.tile_pool(name="xh", bufs=2) as xhp,
            tc.tile_pool(name="xtsb", bufs=2) as xtsbp,
            tc.tile_pool(name="prod", bufs=3) as prodp,
            tc.tile_pool(name="psb", bufs=2) as psbp,
            tc.tile_pool(name="pf", bufs=2) as pfp,
            tc.tile_pool(name="xtps", bufs=3, space="PSUM") as xtpsp,
            tc.tile_pool(name="hps", bufs=2, space="PSUM") as hpsp,
        ):
            ident = singles.tile([128, 128], F32)
            nc.sync.dma_start(out=ident[:], in_=id_in[:, :])
            wext = singles.tile([V, P * NCOL], BF16)
            nc.sync.dma_start(out=wext[:], in_=wext_in[:, :])
            w2e = singles.tile([128, P * NCOL], F32)
            nc.sync.dma_start(out=w2e[:], in_=w2e_in[:, :])

            for bt in range(NBT):
                p_sb = psbp.tile([128, P], F32)
                for half in range(2):
                    xh = xhp.tile([128, 64 * V], F32)
                    nc.sync.dma_start(
                        out=xh[:],
                        in_=x_in[bt * 128 : (bt + 1) * 128,
                                 half * 64 * V : (half + 1) * 64 * V],
                    )
                    # transpose 64 pathway blocks, 4 per PSUM bank, cast to bf16
                    xt_all = xtsbp.tile([128, 64 * 128], BF16)
                    for c in range(16):
                        xt_ps = xtpsp.tile([128, 512], F32)
                        for k in range(4):
                            nc.tensor.transpose(
                                xt_ps[:, k * 128 : (k + 1) * 128],
                                xh[:, (c * 4 + k) * 128 : (c * 4 + k + 1) * 128],
                                ident[:],
                            )
                        nc.scalar.copy(
                            out=xt_all[:, c * 512 : (c + 1) * 512], in_=xt_ps[:]
                        )
                    for gs, G in GROUPS:
                        g2 = G // 2
                        h_ps = hpsp.tile([128, 1024], F32)
                        for j in range(G):
                            pa = half * 64 + gs + j
                            off = (j // g2) * 512 + (j % g2) * NCOL
                            nc.tensor.matmul(
                                h_ps[:, off : off + NCOL],
                                lhsT=xt_all[:, (gs + j) * 128 : (gs + j + 1) * 128],
                                rhs=wext[:, pa * NCOL : (pa + 1) * NCOL],
                                start=True,
                                stop=True,
                            )
                        prod = prodp.tile([128, G * NCOL], F32)
                        h4d = h_ps[:].rearrange("p (b c) -> p b c", b=2)[
                            :, :, : g2 * NCOL
                        ].rearrange("p b (g c) -> p b g c", c=NCOL)
                        w4d = w2e[
                            :, (half * 64 + gs) * NCOL : (half * 64 + gs + G) * NCOL
                        ].rearrange("p (b g c) -> p b g c", b=2, c=NCOL)
                        pr4d = prod[:].rearrange("p (b g c) -> p b g c", b=2, c=NCOL)
                        # prod = max(h, 0) * w2ext   (relu fused with the scale)
                        nc.vector.scalar_tensor_tensor(
                            out=pr4d,
                            in0=h4d,
                            scalar=0.0,
                            in1=w4d,
                            op0=mybir.AluOpType.max,
                            op1=mybir.AluOpType.mult,
                        )
                        nc.vector.tensor_reduce(
                            out=p_sb[:, half * 64 + gs : half * 64 + gs + G],
                            in_=prod[:].rearrange("p (g c) -> p g c", c=NCOL),
                            axis=mybir.AxisListType.X,
                            op=mybir.AluOpType.add,
                        )
                pf = pfp.tile([128, P], F32)
                # final leaky: max(0.2*z2, z2)
                nc.vector.scalar_tensor_tensor(
                    out=pf[:],
                    in0=p_sb[:],
                    scalar=0.2,
                    in1=p_sb[:],
                    op0=mybir.AluOpType.mult,
                    op1=mybir.AluOpType.max,
                )
                nc.sync.dma_start(
                    out=p_out[bt * 128 : (bt + 1) * 128, :], in_=pf[:]
                )
    nc.finalize()
    return nc


def _prep_weights(W1, W2):
    W1T = np.ascontiguousarray(np.transpose(W1, (0, 2, 1)))          # [P,V,W]
    u = 0.2 * np.einsum("pvw,pw->pv", W1T, W2).astype(np.float32)    # [P,V]
    wext = np.concatenate([W1T, u[:, :, None], -u[:, :, None]], axis=2)  # [P,V,66]
    wext = np.ascontiguousarray(np.transpose(wext, (1, 0, 2))).reshape(V, P * NCOL)
    wext_bf = wext.astype(ml_dtypes.bfloat16)
    w2e = np.concatenate(
        [
            0.8 * W2.astype(np.float32),
            np.ones((P, 1), np.float32),
            -np.ones((P, 1), np.float32),
        ],
        axis=1,
    ).reshape(1, P * NCOL)                                            # [1, P*66]
    w2ext = np.ascontiguousarray(np.broadcast_to(w2e, (128, P * NCOL)))
    return wext_bf, w2ext


def kernel(x, W1, W2, gamma, beta, Wd, bd):
    global LAST_RESULTS
    x = np.ascontiguousarray(np.asarray(x, dtype=np.float32))
    W1 = np.asarray(W1, dtype=np.float32)
    W2 = np.asarray(W2, dtype=np.float32)

    if "nc" not in _CACHE:
        _CACHE["nc"] = _build_program()
    nc = _CACHE["nc"]

    wext_bf, w2ext = _prep_weights(W1, W2)
    ident = np.eye(128, dtype=np.float32)
    in_maps = [
        {
            "xs": x[c * BSH : (c + 1) * BSH, :],
            "wext": wext_bf,
            "w2ext": w2ext,
            "ident": ident,
        }
        for c in range(N_CORES)
    ]
    res = run_bass_kernel_spmd(nc, in_maps, list(range(N_CORES)))
    LAST_RESULTS = res

    pvals = np.concatenate(
        [res.results[c]["ps"] for c in range(N_CORES)], axis=0
    ).astype(np.float64)                                              # [B, P]

    mean = pvals.mean(axis=0)
    var = pvals.var(axis=0)
    pn = (pvals - mean) / np.sqrt(var + BN_EPS) * np.asarray(gamma, np.float64) \
        + np.asarray(beta, np.float64)
    pn = pn / np.linalg.norm(pn)
    out = 1.0 / (1.0 + np.exp(-(pn @ np.asarray(Wd, np.float64)
                                + np.asarray(bd, np.float64))))
    return out.astype(np.float32)
